# revision 11
# baseline (speedup 1.0000x reference)
"""Trainium2 Bass kernel: CenterHeadIoU 1x1-conv stack.

Computes, for x = ct_feat [B=32, C=128, N=8192]:
  y = relu(bn(sh_w @ x))                       [B, 64, N]
  z_h = relu(bn_h(head_w1[h] @ y)), h=0..5     [B, 64, N] each
  out = concat_h(head_final_w[h] @ z_h + b_h)  [B, 12, N]

Sharding: data-parallel over batch, 4 batches per core on 8 cores;
weights are tiny and replicated. BN is folded into conv weights/biases
on the host. On device, per 512-column tile:
  mm1: lhsT [128,128] = [W_sh^T | W_sh^T] -> psum y duplicated into
       both partition halves (so head matmuls can use K=128 block-diag)
  act1: relu(y + b) PSUM->SBUF (per-partition bias)
  mm2 (x3): block-diag pair weights [128,128] -> psum z-pair [128,512]
  act2 (x3): relu(z + b), split across ScalarE/VectorE
  mm3 (x3): accumulating matmuls (M=12, zero-padded pair blocks) into one
       dense [12, F] psum bank
  epi: single activation adds final bias into a dense [12, N] per-batch
       output accumulator; 1 DMA out per batch.
All matmuls run as float32r (full-rate fp32 mode, free dim 512).
"""

import os
import sys
import numpy as np

B, C_IN, N, HC = 32, 128, 8192, 64
NCORES = 8
BC = B // NCORES            # batches per core
F = 512                     # free-dim tile = one fp32 PSUM bank
NT = N // F                 # tiles per batch
EPS = 1e-5
HEAD_OUT = [3, 2, 1, 3, 2, 1]        # hm, reg, height, dim, rot, iou
PAIR_OFF = [0, 5, 9]                 # channel offset of pair p in the 12-ch output
PAIR_SZ = [5, 4, 3]

_CACHE = {}
LAST_RESULTS = None
LAST_EXEC_NS = None


def _build_program():
    import concourse.bass as bass
    import concourse.mybir as mybir
    import concourse.tile as tile

    f32 = mybir.dt.float32
    f32r = mybir.dt.float32r
    AF = mybir.ActivationFunctionType
    ALU = mybir.AluOpType

    nc = bass.Bass("TRN2", target_bir_lowering=False, debug=False,
                   num_devices=NCORES)

    x = nc.dram_tensor("x", [BC, C_IN, N], f32r, kind="ExternalInput").ap()
    w1 = nc.dram_tensor("w1", [C_IN, 128], f32r, kind="ExternalInput").ap()
    b1 = nc.dram_tensor("b1", [128, 1], f32, kind="ExternalInput").ap()
    w2 = nc.dram_tensor("w2", [128, 384], f32r, kind="ExternalInput").ap()
    b2 = nc.dram_tensor("b2", [128, 3], f32, kind="ExternalInput").ap()
    w3 = nc.dram_tensor("w3", [128, 36], f32r, kind="ExternalInput").ap()
    b3 = nc.dram_tensor("b3", [128, 1], f32, kind="ExternalInput").ap()
    out = nc.dram_tensor("out", [BC, 12, N], f32, kind="ExternalOutput").ap()

    with tile.TileContext(nc) as tc:
        with (
            tc.tile_pool(name="consts", bufs=1) as cpool,
            tc.tile_pool(name="xin", bufs=4) as xpool,
            tc.tile_pool(name="ysb", bufs=3) as ypool,
            tc.tile_pool(name="zsb", bufs=2) as zpool,
            tc.tile_pool(name="osb", bufs=2) as opool,
            tc.tile_pool(name="py", bufs=2, space="PSUM") as pypool,
            tc.tile_pool(name="pz", bufs=1, space="PSUM") as pzpool,
            tc.tile_pool(name="po", bufs=2, space="PSUM") as popool,
        ):
            w1_t = cpool.tile([C_IN, 128], f32r, name="w1_t")
            nc.sync.dma_start(out=w1_t[:], in_=w1[:])
            b1_t = cpool.tile([128, 1], f32, name="b1_t")
            nc.sync.dma_start(out=b1_t[:], in_=b1[:])
            w2_t = cpool.tile([128, 384], f32r, name="w2_t")
            nc.sync.dma_start(out=w2_t[:], in_=w2[:])
            b2_t = cpool.tile([128, 3], f32, name="b2_t")
            nc.sync.dma_start(out=b2_t[:], in_=b2[:])
            w3_t = cpool.tile([128, 36], f32r, name="w3_t")
            nc.sync.dma_start(out=w3_t[:], in_=w3[:])
            b3_t = cpool.tile([128, 1], f32, name="b3_t")
            nc.sync.dma_start(out=b3_t[:], in_=b3[:])

            for b in range(BC):
                ot = opool.tile([12, N], f32, name="ot", tag="ot")
                for j in range(NT):
                    xt = xpool.tile([C_IN, F], f32r, name="xt", tag="xt")
                    nc.sync.dma_start(out=xt[:], in_=x[b, :, j * F:(j + 1) * F])

                    py = pypool.tile([128, F], f32, name="py", tag="py")
                    nc.tensor.matmul(py[:], w1_t[:],
                                     xt[:], start=True, stop=True)

                    y2 = ypool.tile([128, F], f32r, name="y2", tag="y2")
                    nc.scalar.activation(y2[:], py[:], AF.Relu,
                                         bias=b1_t[:, 0:1], scale=1.0)

                    z_ts = []
                    for p in range(3):
                        pz = pzpool.tile([128, F], f32, name=f"pz{p}",
                                         tag=f"pz{p}")
                        nc.tensor.matmul(pz[:],
                                         w2_t[:, 128 * p:128 * (p + 1)],
                                         y2[:],
                                         start=True, stop=True)
                        zt = zpool.tile([128, F], f32r, name=f"z{p}", tag=f"z{p}")
                        if p == 0:
                            nc.scalar.activation(zt[:], pz[:], AF.Relu,
                                                 bias=b2_t[:, p:p + 1], scale=1.0)
                        else:
                            nc.vector.tensor_scalar(
                                out=zt[:], in0=pz[:],
                                scalar1=b2_t[:, p:p + 1], scalar2=0.0,
                                op0=ALU.add, op1=ALU.max)
                        z_ts.append(zt)

                    # Three accumulating matmuls into one [12, F] bank: each
                    # pair's weight block is padded to M=12 with zeros, so
                    # the PSUM sum scatters each pair into its channel rows.
                    po = popool.tile([12, F], f32, name="po", tag="po")
                    for p in range(3):
                        nc.tensor.matmul(po[:, :],
                                         w3_t[:, 12 * p:12 * (p + 1)],
                                         z_ts[p][:],
                                         start=(p == 0), stop=(p == 2))
                    nc.scalar.activation(ot[:, j * F:(j + 1) * F], po[:, :],
                                         AF.Identity, bias=b3_t[0:12, 0:1],
                                         scale=1.0)
                nc.sync.dma_start(out=out[b, :, :], in_=ot[:, :])
    _split_waits(nc)
    return nc


def _split_waits(nc, cap=1):
    """This container's walrus build rejects instructions carrying more than
    a small number of sync waits (fp32/f32r matmuls: just one). Move excess
    waits onto single-wait Drain carriers inserted before the instruction on
    the same engine — semantically identical (conjunction of waits, in-order
    sequencers)."""
    import concourse.mybir as mybir

    k = 0
    for func in nc.m.functions:
        for bb in func.blocks:
            insts = bb.instructions
            out_insts = []
            changed = False
            for inst in insts:
                si = inst.sync_info
                waits = list(si.on_wait) if si and si.on_wait else []
                if len(waits) > cap:
                    for w in waits[:-cap]:
                        d = mybir.InstDrain(name=f"I-sw{k}", ins=[], outs=[])
                        k += 1
                        d.engine = inst.engine
                        d.sync_info = mybir.SyncInfo(on_wait=[w], on_update=[])
                        nc.register_instruction(d)
                        out_insts.append(d)
                    inst.sync_info = mybir.SyncInfo(
                        on_wait=waits[-cap:],
                        on_update=list(si.on_update) if si.on_update else [])
                    changed = True
                out_insts.append(inst)
            if changed:
                bb.instructions = out_insts


def _get_program():
    if "nc" not in _CACHE:
        _CACHE["nc"] = _build_program()
    return _CACHE["nc"]


def _prep_weights(d):
    """Fold BN into conv weights/biases; pack block-diagonal stationaries."""
    f8 = np.float64

    def g(name):
        return np.asarray(d[name], dtype=f8)

    # shared conv + BN
    s1 = g("sh_g") / np.sqrt(g("sh_var") + EPS)                     # [64]
    W1e = g("sh_w") * s1[:, None]                                   # [64,128]
    b1e = g("sh_b") * s1 + g("sh_beta") - g("sh_mean") * s1         # [64]
    w1 = np.concatenate([W1e.T, W1e.T], axis=1)                     # [128,128]
    b1 = np.tile(b1e, 2)[:, None]                                   # [128,1]

    # head first layers + BN
    s2 = g("head_g1") / np.sqrt(g("head_var1") + EPS)               # [6,64]
    W2e = g("head_w1") * s2[:, :, None]                             # [6,64,64]
    b2e = g("head_b1") * s2 + g("head_beta1") - g("head_mean1") * s2  # [6,64]
    w2 = np.zeros((128, 384), f8)
    b2 = np.zeros((128, 3), f8)
    for p in range(3):
        w2[0:64, 128 * p:128 * p + 64] = W2e[2 * p].T
        w2[64:128, 128 * p + 64:128 * p + 128] = W2e[2 * p + 1].T
        b2[0:64, p] = b2e[2 * p]
        b2[64:128, p] = b2e[2 * p + 1]

    # final convs
    names = ["hm", "reg", "height", "dim", "rot", "iou"]
    Wf = [g(n + "_w") for n in names]
    bf = [g(n + "_b") for n in names]
    w3 = np.zeros((128, 36), f8)
    b3 = np.zeros((128, 1), f8)
    for p in range(3):
        ha, hb = 2 * p, 2 * p + 1
        ca, cb = HEAD_OUT[ha], HEAD_OUT[hb]
        off = PAIR_OFF[p]
        w3[0:64, 12 * p + off:12 * p + off + ca] = Wf[ha].T
        w3[64:128, 12 * p + off + ca:12 * p + off + ca + cb] = Wf[hb].T
        b3[off:off + ca, 0] = bf[ha]
        b3[off + ca:off + ca + cb, 0] = bf[hb]

    c = np.float32
    return {"w1": w1.astype(c), "b1": b1.astype(c), "w2": w2.astype(c),
            "b2": b2.astype(c), "w3": w3.astype(c), "b3": b3.astype(c)}


def _ensure_ntff_hook():
    """Install the antenv.axon_hooks NTFF-profile shim if the container's
    antenv package lacks it (profiling only; never used in grading runs)."""
    try:
        from antenv.axon_hooks import get_axon_ntff_profile_hook  # noqa: F401
        return True
    except ImportError:
        pass
    import contextlib
    import ctypes
    import sys as _sys
    import types

    so_path = "/opt/axon/libaxon_pjrt.so"
    if not os.path.exists(so_path):
        return False
    lib = ctypes.CDLL(so_path)
    if not hasattr(lib, "axon_start_nrt_profile"):
        return False
    lib.axon_start_nrt_profile.argtypes = [ctypes.POINTER(ctypes.c_int64),
                                           ctypes.c_size_t]
    lib.axon_start_nrt_profile.restype = ctypes.c_int64
    lib.axon_stop_nrt_profile.argtypes = [ctypes.c_char_p]
    lib.axon_stop_nrt_profile.restype = ctypes.c_int64

    @contextlib.contextmanager
    def _hook(output_dir, device_ids):
        import jax
        jax.devices()
        if device_ids:
            ids = (ctypes.c_int64 * len(device_ids))(*device_ids)
            rc = lib.axon_start_nrt_profile(ids, len(device_ids))
        else:
            rc = lib.axon_start_nrt_profile(None, 0)
        if rc != 0:
            raise RuntimeError(f"axon_start_nrt_profile rc={rc}")
        try:
            yield
        finally:
            n = lib.axon_stop_nrt_profile(str(output_dir).encode())
            print(f"profile: {n} file(s) written to {output_dir}",
                  file=sys.stderr)

    import antenv
    mod = types.ModuleType("antenv.axon_hooks")
    mod.get_axon_ntff_profile_hook = lambda: _hook
    mod.set_axon_ntff_profile_hook = lambda h: None
    _sys.modules["antenv.axon_hooks"] = mod
    antenv.axon_hooks = mod
    return True


def kernel(**inputs):
    global LAST_RESULTS, LAST_EXEC_NS
    from concourse.bass_utils import run_bass_kernel_spmd

    inputs = {k: np.asarray(v) for k, v in inputs.items()}
    weights = _prep_weights(inputs)

    ct = np.asarray(inputs["ct_feat"], dtype=np.float32)
    xs = ct.reshape(NCORES, BC, C_IN, N)

    in_maps = [dict(weights, x=np.ascontiguousarray(xs[i]))
               for i in range(NCORES)]

    nc = _get_program()
    trace = bool(int(os.environ.get("CK_PROFILE", "0")))
    if trace:
        trace = _ensure_ntff_hook()
    res = run_bass_kernel_spmd(nc, in_maps, list(range(NCORES)), trace=trace)
    LAST_RESULTS = res
    LAST_EXEC_NS = res.exec_time_ns

    out = np.concatenate([np.asarray(res.results[i]["out"])
                          for i in range(NCORES)], axis=0)
    return out.astype(np.float32)


# revision 12
# speedup vs baseline: 1.0751x; 1.0751x over previous
"""Trainium2 Bass kernel: CenterHeadIoU 1x1-conv stack.

Computes, for x = ct_feat [B=32, C=128, N=8192]:
  y = relu(bn(sh_w @ x))                       [B, 64, N]
  z_h = relu(bn_h(head_w1[h] @ y)), h=0..5     [B, 64, N] each
  out = concat_h(head_final_w[h] @ z_h + b_h)  [B, 12, N]

Sharding: data-parallel over batch, 4 batches per core on 8 cores;
weights are tiny and replicated. BN is folded into conv weights/biases
on the host. On device, per 512-column tile:
  mm1: lhsT [128,128] = [W_sh^T | W_sh^T] -> psum y duplicated into
       both partition halves (so head matmuls can use K=128 block-diag)
  act1: relu(y + b) PSUM->SBUF (per-partition bias)
  mm2 (x3): block-diag pair weights [128,128] -> psum z-pair [128,512]
  act2 (x3): relu(z + b), split across ScalarE/VectorE
  mm3 (x3): accumulating matmuls (M=12, zero-padded pair blocks) into one
       dense [12, F] psum bank
  epi: single activation adds final bias into a dense [12, N] per-batch
       output accumulator; 1 DMA out per batch.
All matmuls run as float32r (full-rate fp32 mode, free dim 512).
"""

import os
import sys
import numpy as np

B, C_IN, N, HC = 32, 128, 8192, 64
NCORES = 8
BC = B // NCORES            # batches per core
F = 512                     # free-dim tile = one fp32 PSUM bank
NT = N // F                 # tiles per batch
EPS = 1e-5
HEAD_OUT = [3, 2, 1, 3, 2, 1]        # hm, reg, height, dim, rot, iou
PAIR_OFF = [0, 5, 9]                 # channel offset of pair p in the 12-ch output
PAIR_SZ = [5, 4, 3]

_CACHE = {}
LAST_RESULTS = None
LAST_EXEC_NS = None


def _build_program():
    import concourse.bass as bass
    import concourse.mybir as mybir
    import concourse.tile as tile

    f32 = mybir.dt.float32
    f32r = mybir.dt.float32r
    AF = mybir.ActivationFunctionType
    ALU = mybir.AluOpType

    nc = bass.Bass("TRN2", target_bir_lowering=False, debug=False,
                   num_devices=NCORES)

    x = nc.dram_tensor("x", [BC, C_IN, N], f32r, kind="ExternalInput").ap()
    w1 = nc.dram_tensor("w1", [C_IN, 128], f32r, kind="ExternalInput").ap()
    b1 = nc.dram_tensor("b1", [128, 1], f32, kind="ExternalInput").ap()
    w2 = nc.dram_tensor("w2", [128, 384], f32r, kind="ExternalInput").ap()
    b2 = nc.dram_tensor("b2", [128, 3], f32, kind="ExternalInput").ap()
    w3 = nc.dram_tensor("w3", [128, 36], f32r, kind="ExternalInput").ap()
    b3 = nc.dram_tensor("b3", [128, 1], f32, kind="ExternalInput").ap()
    out = nc.dram_tensor("out", [BC, 12, N], f32, kind="ExternalOutput").ap()

    with tile.TileContext(nc) as tc:
        with (
            tc.tile_pool(name="consts", bufs=1) as cpool,
            tc.tile_pool(name="xin", bufs=4) as xpool,
            tc.tile_pool(name="ysb", bufs=3) as ypool,
            tc.tile_pool(name="zsb", bufs=2) as zpool,
            tc.tile_pool(name="osb", bufs=2) as opool,
            tc.tile_pool(name="py", bufs=2, space="PSUM") as pypool,
            tc.tile_pool(name="pz", bufs=1, space="PSUM") as pzpool,
            tc.tile_pool(name="po", bufs=2, space="PSUM") as popool,
        ):
            w1_t = cpool.tile([C_IN, 128], f32r, name="w1_t")
            nc.sync.dma_start(out=w1_t[:], in_=w1[:])
            b1_t = cpool.tile([128, 1], f32, name="b1_t")
            nc.sync.dma_start(out=b1_t[:], in_=b1[:])
            w2_t = cpool.tile([128, 384], f32r, name="w2_t")
            nc.sync.dma_start(out=w2_t[:], in_=w2[:])
            b2_t = cpool.tile([128, 3], f32, name="b2_t")
            nc.sync.dma_start(out=b2_t[:], in_=b2[:])
            w3_t = cpool.tile([128, 36], f32r, name="w3_t")
            nc.sync.dma_start(out=w3_t[:], in_=w3[:])
            b3_t = cpool.tile([128, 1], f32, name="b3_t")
            nc.sync.dma_start(out=b3_t[:], in_=b3[:])

            for b in range(BC):
                ot = opool.tile([12, N], f32, name="ot", tag="ot")
                for j in range(NT):
                    xt = xpool.tile([C_IN, F], f32r, name="xt", tag="xt")
                    nc.sync.dma_start(out=xt[:], in_=x[b, :, j * F:(j + 1) * F])

                    py = pypool.tile([128, F], f32, name="py", tag="py")
                    nc.tensor.matmul(py[:], w1_t[:],
                                     xt[:], start=True, stop=True)

                    y2 = ypool.tile([128, F], f32r, name="y2", tag="y2")
                    nc.scalar.activation(y2[:], py[:], AF.Relu,
                                         bias=b1_t[:, 0:1], scale=1.0)

                    z_ts = []
                    for p in range(3):
                        pz = pzpool.tile([128, F], f32, name=f"pz{p}",
                                         tag=f"pz{p}")
                        nc.tensor.matmul(pz[:],
                                         w2_t[:, 128 * p:128 * (p + 1)],
                                         y2[:],
                                         start=True, stop=True)
                        zt = zpool.tile([128, F], f32r, name=f"z{p}", tag=f"z{p}")
                        if p == 0:
                            nc.scalar.activation(zt[:], pz[:], AF.Relu,
                                                 bias=b2_t[:, p:p + 1], scale=1.0)
                        else:
                            nc.vector.tensor_scalar(
                                out=zt[:], in0=pz[:],
                                scalar1=b2_t[:, p:p + 1], scalar2=0.0,
                                op0=ALU.add, op1=ALU.max)
                        z_ts.append(zt)

                    # Three accumulating matmuls into one [12, F] bank: each
                    # pair's weight block is padded to M=12 with zeros, so
                    # the PSUM sum scatters each pair into its channel rows.
                    po = popool.tile([12, F], f32, name="po", tag="po")
                    for p in range(3):
                        nc.tensor.matmul(po[:, :],
                                         w3_t[:, 12 * p:12 * (p + 1)],
                                         z_ts[p][:],
                                         start=(p == 0), stop=(p == 2))
                    nc.scalar.activation(ot[:, j * F:(j + 1) * F], po[:, :],
                                         AF.Identity, bias=b3_t[0:12, 0:1],
                                         scale=1.0)
                nc.sync.dma_start(out=out[b, :, :], in_=ot[:, :])
    _split_waits(nc)
    return nc


def _split_waits(nc, cap=1):
    """This container's walrus build rejects instructions carrying more than
    a small number of sync waits (fp32/f32r matmuls: just one). Move excess
    waits onto single-wait Drain carriers inserted before the instruction on
    the same engine — semantically identical (conjunction of waits, in-order
    sequencers)."""
    import concourse.mybir as mybir

    k = 0
    for func in nc.m.functions:
        for bb in func.blocks:
            insts = bb.instructions
            out_insts = []
            changed = False
            for inst in insts:
                si = inst.sync_info
                waits = list(si.on_wait) if si and si.on_wait else []
                if len(waits) > cap:
                    for w in waits[:-cap]:
                        d = mybir.InstNoOp(name=f"I-sw{k}", ins=[], outs=[])
                        k += 1
                        d.engine = inst.engine
                        d.sync_info = mybir.SyncInfo(on_wait=[w], on_update=[])
                        nc.register_instruction(d)
                        out_insts.append(d)
                    inst.sync_info = mybir.SyncInfo(
                        on_wait=waits[-cap:],
                        on_update=list(si.on_update) if si.on_update else [])
                    changed = True
                out_insts.append(inst)
            if changed:
                bb.instructions = out_insts


def _get_program():
    if "nc" not in _CACHE:
        _CACHE["nc"] = _build_program()
    return _CACHE["nc"]


def _prep_weights(d):
    """Fold BN into conv weights/biases; pack block-diagonal stationaries."""
    f8 = np.float64

    def g(name):
        return np.asarray(d[name], dtype=f8)

    # shared conv + BN
    s1 = g("sh_g") / np.sqrt(g("sh_var") + EPS)                     # [64]
    W1e = g("sh_w") * s1[:, None]                                   # [64,128]
    b1e = g("sh_b") * s1 + g("sh_beta") - g("sh_mean") * s1         # [64]
    w1 = np.concatenate([W1e.T, W1e.T], axis=1)                     # [128,128]
    b1 = np.tile(b1e, 2)[:, None]                                   # [128,1]

    # head first layers + BN
    s2 = g("head_g1") / np.sqrt(g("head_var1") + EPS)               # [6,64]
    W2e = g("head_w1") * s2[:, :, None]                             # [6,64,64]
    b2e = g("head_b1") * s2 + g("head_beta1") - g("head_mean1") * s2  # [6,64]
    w2 = np.zeros((128, 384), f8)
    b2 = np.zeros((128, 3), f8)
    for p in range(3):
        w2[0:64, 128 * p:128 * p + 64] = W2e[2 * p].T
        w2[64:128, 128 * p + 64:128 * p + 128] = W2e[2 * p + 1].T
        b2[0:64, p] = b2e[2 * p]
        b2[64:128, p] = b2e[2 * p + 1]

    # final convs
    names = ["hm", "reg", "height", "dim", "rot", "iou"]
    Wf = [g(n + "_w") for n in names]
    bf = [g(n + "_b") for n in names]
    w3 = np.zeros((128, 36), f8)
    b3 = np.zeros((128, 1), f8)
    for p in range(3):
        ha, hb = 2 * p, 2 * p + 1
        ca, cb = HEAD_OUT[ha], HEAD_OUT[hb]
        off = PAIR_OFF[p]
        w3[0:64, 12 * p + off:12 * p + off + ca] = Wf[ha].T
        w3[64:128, 12 * p + off + ca:12 * p + off + ca + cb] = Wf[hb].T
        b3[off:off + ca, 0] = bf[ha]
        b3[off + ca:off + ca + cb, 0] = bf[hb]

    c = np.float32
    return {"w1": w1.astype(c), "b1": b1.astype(c), "w2": w2.astype(c),
            "b2": b2.astype(c), "w3": w3.astype(c), "b3": b3.astype(c)}


def _ensure_ntff_hook():
    """Install the antenv.axon_hooks NTFF-profile shim if the container's
    antenv package lacks it (profiling only; never used in grading runs)."""
    try:
        from antenv.axon_hooks import get_axon_ntff_profile_hook  # noqa: F401
        return True
    except ImportError:
        pass
    import contextlib
    import ctypes
    import sys as _sys
    import types

    so_path = "/opt/axon/libaxon_pjrt.so"
    if not os.path.exists(so_path):
        return False
    lib = ctypes.CDLL(so_path)
    if not hasattr(lib, "axon_start_nrt_profile"):
        return False
    lib.axon_start_nrt_profile.argtypes = [ctypes.POINTER(ctypes.c_int64),
                                           ctypes.c_size_t]
    lib.axon_start_nrt_profile.restype = ctypes.c_int64
    lib.axon_stop_nrt_profile.argtypes = [ctypes.c_char_p]
    lib.axon_stop_nrt_profile.restype = ctypes.c_int64

    @contextlib.contextmanager
    def _hook(output_dir, device_ids):
        import jax
        jax.devices()
        if device_ids:
            ids = (ctypes.c_int64 * len(device_ids))(*device_ids)
            rc = lib.axon_start_nrt_profile(ids, len(device_ids))
        else:
            rc = lib.axon_start_nrt_profile(None, 0)
        if rc != 0:
            raise RuntimeError(f"axon_start_nrt_profile rc={rc}")
        try:
            yield
        finally:
            n = lib.axon_stop_nrt_profile(str(output_dir).encode())
            print(f"profile: {n} file(s) written to {output_dir}",
                  file=sys.stderr)

    import antenv
    mod = types.ModuleType("antenv.axon_hooks")
    mod.get_axon_ntff_profile_hook = lambda: _hook
    mod.set_axon_ntff_profile_hook = lambda h: None
    _sys.modules["antenv.axon_hooks"] = mod
    antenv.axon_hooks = mod
    return True


def kernel(**inputs):
    global LAST_RESULTS, LAST_EXEC_NS
    from concourse.bass_utils import run_bass_kernel_spmd

    inputs = {k: np.asarray(v) for k, v in inputs.items()}
    weights = _prep_weights(inputs)

    ct = np.asarray(inputs["ct_feat"], dtype=np.float32)
    xs = ct.reshape(NCORES, BC, C_IN, N)

    in_maps = [dict(weights, x=np.ascontiguousarray(xs[i]))
               for i in range(NCORES)]

    nc = _get_program()
    trace = bool(int(os.environ.get("CK_PROFILE", "0")))
    if trace:
        trace = _ensure_ntff_hook()
    res = run_bass_kernel_spmd(nc, in_maps, list(range(NCORES)), trace=trace)
    LAST_RESULTS = res
    LAST_EXEC_NS = res.exec_time_ns

    out = np.concatenate([np.asarray(res.results[i]["out"])
                          for i in range(NCORES)], axis=0)
    return out.astype(np.float32)
